# revision 1
# baseline (speedup 1.0000x reference)
"""Trainium2 Bass kernel for nn_Metalayer_sub_62869731279045.

Math: the edge list from the oracle's setup_inputs() is the structured 1-D
KNN=2 neighbor graph, so C = I + Delta and Km are pentadiagonal (offsets
-2,-1,+1,+2) with |Delta| entries <= 0.1 (0.1*tanh).  We never form C^-1
or expm densely:

  Uz = expm(1j*wh*C^-1(B C + K)) @ U0
     = e^{i*theta} * sum_k t_k,  t_k = (i T') t_{k-1} / k,  t_0 = U0
  T' v = wh * C^-1 (G v) - theta v,     G = B C + K   (pentadiagonal)
  C^-1 w ~= sum_{j=0..J} (-Delta)^j w                 (Neumann)

With theta ~ wh*k*mean(neff) hardcoded the shifted operator has small norm;
KT=8 Taylor terms with JN=4 Neumann give ~1.3e-4 relative error vs fp64.

Layout: length-2048 real vectors are [128 partitions, 16] free-minor
(flat i = 16*p + f).  Complex chain vectors are [128, 40] tiles:
re = pad(2)|data(16)|pad(2) at cols 0..19, im at cols 20..39.  One
pentadiagonal matvec = 2 PE shift-matmuls refresh the halo pads from
neighboring partitions, then one DVE 4-D windowed multiply against 5
stacked coefficient planes and one segmented reduce.

All 8 cores run the same single-core program on identical inputs (the
chain is a serial dependency; collectives would cost more than they save).
Core 0's output is returned.
"""

import os
import sys
import numpy as np

for _p in ("/opt/trn_rl_repo",):
    if _p not in sys.path:
        sys.path.insert(0, _p)

N = 2048
RES = 32
H = 64
E = 8186
K_WAVE = 2.0 * np.pi / 1.55
WH = 0.75
DX = 1.0 / RES
THETA = 6.234  # ~ WH*K_WAVE*mean(neff); pure series shift, nearby value is fine
JN = 4         # Neumann order for C^-1
KT = 8         # Taylor order for expm action

# (offset o, i0 = first valid row index, L = edge count, e0 = edge-array start)
BANDS = [(-2, 2, 2046, 0), (-1, 1, 2047, 2046), (1, 0, 2047, 4093), (2, 0, 2046, 6140)]
PLANE = {-2: 0, -1: 1, 1: 3, 2: 4}  # coefficient plane s holds shift o = s-2

_CACHE = {}


def _build():
    from contextlib import ExitStack

    import concourse.bass as bass
    import concourse.mybir as mybir
    from concourse import bacc, tile

    f32 = mybir.dt.float32
    bf16 = mybir.dt.bfloat16
    f32r = mybir.dt.float32r
    AF = mybir.ActivationFunctionType
    ALU = mybir.AluOpType

    use_f32r = os.environ.get("KERNEL_F32R", "0") == "1"
    phase = int(os.environ.get("KERNEL_PHASE", "9"))
    repeat = int(os.environ.get("KERNEL_REPEAT", "1"))

    nc = bacc.Bacc("TRN2", target_bir_lowering=False, debug=False, num_devices=8)

    def Par(name, shape):
        return nc.declare_dram_parameter(name, list(shape), f32, isOutput=False)

    hs_d = Par("hs", [N])
    dis_d = Par("dis", [8192])
    e0c_d = Par("e0c", [N * RES])
    w = {}
    for pre in ("n", "c", "k", "e"):
        fin = 1 if pre in ("n", "e") else 3
        fout = RES if pre == "e" else 1
        w[pre + "W1"] = Par(pre + "W1", [fin, H])
        w[pre + "W2"] = Par(pre + "W2", [H, H])
        w[pre + "W3"] = Par(pre + "W3", [H, fout])
        w[pre + "b1"] = Par(pre + "b1", [H])
        w[pre + "b2"] = Par(pre + "b2", [H])
        w[pre + "b3"] = Par(pre + "b3", [fout])
    sdn_d = Par("sdn", [128, 128])
    sup_d = Par("sup", [128, 128])
    mask_d = Par("bmask", [128, 64])
    eysbuf = nc.dram_tensor("eysbuf", [RES, N], f32)
    out_d = nc.declare_dram_parameter("out", [N * RES, 2], f32, isOutput=True)

    def mmr(psum_ap, lhsT_ap, rhs_ap):
        if use_f32r:
            nc.tensor.matmul(psum_ap, lhsT_ap.bitcast(f32r), rhs_ap.bitcast(f32r))
        else:
            nc.tensor.matmul(psum_ap, lhsT_ap, rhs_ap)

    def win4(t):
        """[p, h, f, s] overlapping 5-shift window over a [128,40] padded tile."""
        return bass.AP(t.tensor, t.offset, [[40, 128], [20, 2], [1, 16], [1, 5]])

    def planes4(t):
        """[p, h, f, s] view of a [128,160] coefficient tile."""
        return bass.AP(t.tensor, t.offset, [[160, 128], [80, 2], [1, 16], [16, 5]])

    def vdata(t):
        """[p, h, f] view of the 32 data columns of a [128,40] padded tile."""
        return bass.AP(t.tensor, t.offset + 2, [[40, 128], [20, 2], [1, 16]])

    def dre(t):
        return bass.AP(t.tensor, t.offset + 2, [[40, 128], [1, 16]])

    def dim_(t):
        return bass.AP(t.tensor, t.offset + 22, [[40, 128], [1, 16]])

    l3count = [0]

    def emit(tc, ctx, pools):
        (consts, big1, big2, ps_big, ps_row, ps_sm, fm, vec, glue) = pools
        dma_engines = [nc.sync, nc.gpsimd, nc.scalar]
        dma_i = [0]

        def dmae(out_ap, in_ap):
            e = dma_engines[dma_i[0] % len(dma_engines)]
            dma_i[0] += 1
            e.dma_start(out_ap, in_ap)

        # ---------------- constants / weights ----------------
        hs_row = consts.tile([1, N], f32, tag="hsrow")
        dmae(hs_row[:], hs_d[None, :])
        sdn = consts.tile([128, 128], f32, tag="sdn")
        dmae(sdn[:], sdn_d[:])
        sup = consts.tile([128, 128], f32, tag="sup")
        dmae(sup[:], sup_d[:])

        def load_w(name, shape):
            t = consts.tile(list(shape), f32, tag=name)
            dmae(t[:], w[name][:])
            return t

        def load_b(name):
            t = consts.tile([H, 1], f32, tag=name)
            dmae(t[:], w[name][:, None])
            return t

        def load_w3x(name3, nameb, fout):
            # pad single-column weights to 2 columns: M=1 fp32 matmuls
            # produce garbage on TRN2 hardware (M>=2 works)
            cols = max(fout, 2)
            t = consts.tile([H + 1, cols], f32, tag=name3 + "x")
            if fout == 1:
                nc.vector.memset(t[:, 1:2], 0.0)
                dmae(t[0:H, 0:1], w[name3][:])
                dmae(t[H : H + 1, 0:1], w[nameb][:, None])
            else:
                dmae(t[0:H, :], w[name3][:])
                dmae(t[H : H + 1, :], w[nameb][None, :])
            return t

        def to_bf16(t, shape, tag, base=0):
            tb = consts.tile(list(shape), bf16, tag=tag)
            if base:
                nc.vector.tensor_copy(tb[base:, :], t)
                return tb[base:, :]
            nc.vector.tensor_copy(tb[:], t[:])
            return tb

        nW1, nW2f = load_w("nW1", (1, H)), load_w("nW2", (H, H))
        nW2 = to_bf16(nW2f, (H, H), "nW2b")
        nb1, nb2 = load_b("nb1"), load_b("nb2")
        nW3x = to_bf16(load_w3x("nW3", "nb3", 1), (H + 1, 2), "nW3xb")
        eW1, eW2f = load_w("eW1", (1, H)), load_w("eW2", (H, H))
        eW2 = to_bf16(eW2f, (H, H), "eW2b")
        eb1, eb2 = load_b("eb1"), load_b("eb2")
        eW3x = to_bf16(load_w3x("eW3", "eb3", RES), (H + 1, RES), "eW3xb")
        W1ck = consts.tile([3, 128], f32, tag="W1ck")
        dmae(W1ck[:, 0:H], w["cW1"][:])
        dmae(W1ck[:, H:128], w["kW1"][:])
        b1ck = consts.tile([128, 1], f32, tag="b1ck")
        dmae(b1ck[0:H, :], w["cb1"][:, None])
        dmae(b1ck[H:128, :], w["kb1"][:, None])
        cW2f = load_w("cW2", (H, H))
        cW2 = to_bf16(cW2f, (H, H), "cW2b")
        kW2t = consts.tile([128, H], f32, tag="kW2")
        dmae(kW2t[H:128, :], w["kW2"][:])
        kW2 = to_bf16(kW2t[H:128, :], (128, H), "kW2b", base=H)
        cb2, kb2 = load_b("cb2"), load_b("kb2")
        cW3x = to_bf16(load_w3x("cW3", "cb3", 1), (H + 1, 2), "cW3xb")
        kW3x = to_bf16(load_w3x("kW3", "kb3", 1), (H + 1, 2), "kW3xb")
        bmask = consts.tile([128, 64], f32, tag="bmask")
        dmae(bmask[:], mask_d[:])
        e0c_fm = consts.tile([128, 16 * RES], f32, tag="e0cfm")
        dmae(e0c_fm[:], e0c_d[:].rearrange("(p x) -> p x", p=128))

        vcopy = nc.vector.tensor_copy

        def scopy(o, i):
            nc.scalar.activation(o, i, AF.Copy)

        def layer1(W1t, b1t, npart, tag):
            h1 = big1.tile([npart, N], bf16, tag=tag)
            for q in range(4):
                ps = ps_big.tile([npart, 512], f32, tag="ps")
                mmr(ps[:], W1t[:], hs_row[:, bass.ts(q, 512)])
                nc.scalar.activation(
                    h1[:, bass.ts(q, 512)], ps[:], AF.Relu, bias=b1t[:]
                )
            return h1

        def layer2(pool, h1, src0, W2ap, b2t, tag):
            h2 = pool.tile([H + 1, N], bf16, tag=tag)
            nc.gpsimd.memset(h2[H : H + 1, :], 1.0)
            for q in range(4):
                ps = ps_big.tile([H, 512], f32, tag="ps")
                nc.tensor.matmul(ps[:], W2ap, h1[src0 : src0 + H, bass.ts(q, 512)])
                nc.scalar.activation(
                    h2[0:H, bass.ts(q, 512)], ps[:], AF.Relu, bias=b2t[:]
                )
            return h2

        def layer3_to_fm(W3xt, h2, fm_tag, copy_eng):
            row = big2.tile([1, N], f32, tag="l3row")
            for q in range(4):
                ps = ps_row.tile([2, 512], f32, tag="psrow")
                nc.tensor.matmul(ps[:], W3xt[:], h2[:, bass.ts(q, 512)])
                copy_eng(row[:, bass.ts(q, 512)], ps[0:1, :])
            l3count[0] += 1
            dbuf = nc.dram_tensor(f"l3buf{l3count[0]}", [1, N], f32)
            dmae(dbuf[:], row[:])
            t = fm.tile([128, 16], f32, tag=fm_tag)
            dmae(t[:], dbuf[0, :].rearrange("(p f) -> p f", p=128))
            return t

        if phase == 14:
            hfm = fm.tile([128, 16], f32, tag="hfm")
            nc.sync.dma_start(hfm[:], hs_row[0, :].rearrange("(p f) -> p f", p=128))
            nc.sync.dma_start(bass.AP(out_d, 0, [[16, 128], [1, 16]]), hfm[:])
            return
        # ---------------- node MLP -> Bd ----------------
        h1n = layer1(nW1, nb1, H, "h1n")
        h2n = layer2(big1, h1n, 0, nW2[:], nb2, "h2n")
        Bd = layer3_to_fm(nW3x, h2n, "Bd", vcopy)
        if phase == 13:
            return
        if phase == 11:
            nc.sync.dma_start(bass.AP(out_d, 0, [[16, 128], [1, 16]]), Bd[:])
            return
        if phase == 12:
            nc.sync.dma_start(
                bass.AP(out_d, 0, [[64, 64], [1, 64]]), h2n[0:64, 0:64]
            )
            return
        tb = fm.tile([128, 16], f32, tag="tb")
        nc.scalar.activation(tb[:], Bd[:], AF.Tanh)
        nc.vector.tensor_scalar(
            Bd[:], tb[:], 0.5 * K_WAVE, 2.0 * K_WAVE, ALU.mult, op1=ALU.add
        )
        if phase == 1:
            nc.sync.dma_start(bass.AP(out_d, 0, [[16, 128], [1, 16]]), Bd[:])
            return

        # ---------------- e MLP -> Eys (free-minor, r-inner) ----------------
        h1e = layer1(eW1, eb1, H, "h1e")
        h2e = layer2(big1, h1e, 0, eW2[:], eb2, "h2e")
        eys_rows = big1.tile([RES, N], f32, tag="eysrows")
        for q in range(4):
            ps = ps_big.tile([RES, 512], f32, tag="ps")
            nc.tensor.matmul(ps[:], eW3x[:], h2e[:, bass.ts(q, 512)])
            nc.vector.tensor_copy(eys_rows[:, bass.ts(q, 512)], ps[:])
        dmae(eysbuf[:], eys_rows[:])
        eys_fm = consts.tile([128, 16 * RES], f32, tag="eysfm")
        for r in range(RES):
            dmae(
                bass.AP(eys_fm.tensor, eys_fm.offset + r, [[512, 128], [32, 16]]),
                bass.AP(eysbuf, r * N, [[16, 128], [1, 16]]),
            )
        if phase == 2:
            nc.sync.dma_start(
                bass.AP(out_d, 0, [[512, 128], [1, 512]]), eys_fm[:]
            )
            return

        # ---------------- U0 ----------------
        prod0 = consts.tile([128, 16 * RES], f32, tag="u0prod")
        nc.vector.tensor_mul(prod0[:], eys_fm[:], e0c_fm[:])
        u0 = fm.tile([128, 16], f32, tag="u0")
        nc.vector.reduce_sum(
            u0[:],
            prod0[:].rearrange("p (f r) -> p f r", r=RES),
            axis=mybir.AxisListType.X,
        )
        if phase == 3:
            nc.sync.dma_start(bass.AP(out_d, 0, [[16, 128], [1, 16]]), u0[:])
            return

        # ---------------- edge MLPs -> coefficient planes ----------------
        Gpl = consts.tile([128, 160], f32, tag="Gpl")
        Dpl = consts.tile([128, 160], f32, tag="Dpl")
        nc.vector.memset(Dpl[:, 32:48], 0.0)         # Delta diag plane = 0
        nc.vector.tensor_copy(Gpl[:, 32:48], Bd[:])  # G diag plane = Bd
        for o, i0, L, e0 in BANDS:
            xt = big2.tile([3, N], f32, tag="xt")
            nc.vector.memset(xt[:, 0:2], 0.0)
            nc.vector.memset(xt[:, N - 2 : N], 0.0)
            dmae(xt[0:1, i0 : i0 + L], hs_d[None, i0 : i0 + L])
            dmae(xt[1:2, i0 : i0 + L], hs_d[None, i0 + o : i0 + o + L])
            dmae(xt[2:3, i0 : i0 + L], dis_d[None, e0 : e0 + L])
            h1 = big2.tile([128, N], bf16, tag="h1ck")
            for q in range(4):
                ps = ps_big.tile([128, 512], f32, tag="ps")
                mmr(ps[:], W1ck[:], xt[:, bass.ts(q, 512)])
                nc.scalar.activation(
                    h1[:, bass.ts(q, 512)], ps[:], AF.Relu, bias=b1ck[:]
                )
            h2c = layer2(big2, h1, 0, cW2[:], cb2, "h2c")
            h2k = layer2(big2, h1, H, kW2, kb2, "h2k")
            cpre = layer3_to_fm(cW3x, h2c, "cpre", vcopy)
            kpre = layer3_to_fm(kW3x, h2k, "kpre", vcopy)
            s = PLANE[o]
            tc_t = fm.tile([128, 16], f32, tag="tc")
            tk_t = fm.tile([128, 16], f32, tag="tk")
            nc.scalar.activation(tc_t[:], cpre[:], AF.Tanh)
            nc.scalar.activation(tk_t[:], kpre[:], AF.Tanh)
            bi = BANDS.index((o, i0, L, e0))
            msk = bmask[:, 16 * bi : 16 * (bi + 1)]
            nc.vector.scalar_tensor_tensor(
                Dpl[:, 16 * s : 16 * (s + 1)], tc_t[:], -0.1, msk, ALU.mult, ALU.mult
            )
            gm = fm.tile([128, 16], f32, tag="gm")
            nc.vector.tensor_mul(gm[:], tc_t[:], Bd[:])
            tks = fm.tile([128, 16], f32, tag="tks")
            nc.vector.tensor_scalar(
                tks[:], tk_t[:], 0.1 * K_WAVE, 0.0, ALU.mult, op1=ALU.add
            )
            gtmp = fm.tile([128, 16], f32, tag="gtmp")
            nc.vector.scalar_tensor_tensor(
                gtmp[:], gm[:], 0.1, tks[:], ALU.mult, ALU.add
            )
            nc.vector.tensor_mul(Gpl[:, 16 * s : 16 * (s + 1)], gtmp[:], msk)
        nc.vector.tensor_copy(Gpl[:, 80:160], Gpl[:, 0:80])
        nc.vector.tensor_copy(Dpl[:, 80:160], Dpl[:, 0:80])
        if phase == 4:
            nc.sync.dma_start(bass.AP(out_d, 0, [[160, 128], [1, 160]]), Gpl[:])
            nc.sync.dma_start(bass.AP(out_d, 20480, [[160, 128], [1, 160]]), Dpl[:])
            return

        # ---------------- chain ----------------
        def emit_matvec(v, coeff):
            """w = pentadiagonal(coeff) @ v; fills v's halo pads in place."""
            psh = ps_sm.tile([128, 8], f32, tag="psh")
            vv = v[:].rearrange("p (h c) -> p h c", h=2)
            nc.tensor.matmul(psh[:, 0:4], sup[:], vv[:, :, 16:18])
            nc.tensor.matmul(psh[:, 4:8], sdn[:], vv[:, :, 2:4])
            # one copy fills all four halo pairs: sides x halves x 2 cols
            nc.vector.tensor_copy(
                bass.AP(v.tensor, v.offset, [[40, 128], [18, 2], [20, 2], [1, 2]]),
                bass.AP(psh.tensor, psh.offset, [[8, 128], [4, 2], [2, 2], [1, 2]]),
            )
            pr = glue.tile([128, 160], f32, tag="prod")
            pr4 = pr[:].rearrange("p (h f s) -> p h f s", h=2, f=16)
            nc.vector.tensor_tensor(pr4, win4(v), planes4(coeff), ALU.mult)
            w_t = vec.tile([128, 40], f32, tag="vec")
            nc.vector.reduce_sum(vdata(w_t), pr4, axis=mybir.AxisListType.X)
            return w_t

        t_cur = vec.tile([128, 40], f32, tag="vec")
        nc.vector.memset(t_cur[:], 0.0)
        nc.vector.tensor_scalar(dre(t_cur), u0[:], DX, 0.0, ALU.mult, op1=ALU.add)
        s_re = glue.tile([128, 16], f32, tag="sre")
        s_im = glue.tile([128, 16], f32, tag="sim")
        nc.vector.tensor_scalar(s_re[:], u0[:], DX, 0.0, ALU.mult, op1=ALU.add)
        nc.vector.memset(s_im[:], 0.0)

        for k in range(1, KT + 1):
            x = emit_matvec(t_cur, Gpl)
            u = x
            for j in range(JN):
                u = emit_matvec(u, Dpl)
                nc.vector.tensor_tensor(vdata(x), vdata(x), vdata(u), ALU.add)
            # z = wh*x - theta*t;  t_next = i*z/k;  s += t_next
            pre = glue.tile([128, 32], f32, tag="pre")
            pre3 = pre[:].rearrange("p (h f) -> p h f", h=2)
            nc.vector.tensor_scalar(
                pre3, vdata(t_cur), THETA, 0.0, ALU.mult, op1=ALU.add
            )
            zz = glue.tile([128, 32], f32, tag="zz")
            zz3 = zz[:].rearrange("p (h f) -> p h f", h=2)
            nc.vector.scalar_tensor_tensor(
                zz3, vdata(x), WH, pre3, ALU.mult, ALU.subtract
            )
            t_next = vec.tile([128, 40], f32, tag="vec")
            nc.vector.tensor_scalar(
                dre(t_next), zz[:, 16:32], -1.0 / k, 0.0, ALU.mult, op1=ALU.add
            )
            nc.vector.tensor_scalar(
                dim_(t_next), zz[:, 0:16], 1.0 / k, 0.0, ALU.mult, op1=ALU.add
            )
            nc.vector.tensor_tensor(s_re[:], s_re[:], dre(t_next), ALU.add)
            nc.vector.tensor_tensor(s_im[:], s_im[:], dim_(t_next), ALU.add)
            t_cur = t_next

        # ---------------- Uz = e^{i theta} s;  En = Uz * Eys ----------------
        cth, sth = float(np.cos(THETA)), float(np.sin(THETA))
        uzr = fm.tile([128, 16], f32, tag="uzr")
        uzi = fm.tile([128, 16], f32, tag="uzi")
        p1 = glue.tile([128, 16], f32, tag="p1")
        nc.vector.tensor_scalar(p1[:], s_im[:], sth, 0.0, ALU.mult, op1=ALU.add)
        nc.vector.scalar_tensor_tensor(
            uzr[:], s_re[:], cth, p1[:], ALU.mult, ALU.subtract
        )
        p2 = glue.tile([128, 16], f32, tag="p2")
        nc.vector.tensor_scalar(p2[:], s_re[:], sth, 0.0, ALU.mult, op1=ALU.add)
        nc.vector.scalar_tensor_tensor(uzi[:], s_im[:], cth, p2[:], ALU.mult, ALU.add)
        en_re = consts.tile([128, 16 * RES], f32, tag="enre")
        en_im = consts.tile([128, 16 * RES], f32, tag="enim")
        for dst, uz in ((en_re, uzr), (en_im, uzi)):
            nc.vector.tensor_tensor(
                dst[:].rearrange("p (f r) -> p f r", r=RES),
                eys_fm[:].rearrange("p (f r) -> p f r", r=RES),
                bass.AP(uz.tensor, uz.offset, [[16, 128], [1, 16], [0, 32]]),
                ALU.mult,
            )
        for half in range(2):
            pa, po = 64 * half, 64 * half * 1024
            nc.sync.dma_start(
                bass.AP(out_d, po, [[1024, 64], [2, 512]]), en_re[pa : pa + 64, :]
            )
            nc.sync.dma_start(
                bass.AP(out_d, po + 1, [[1024, 64], [2, 512]]), en_im[pa : pa + 64, :]
            )

    with tile.TileContext(nc) as tc:
        ctx = ExitStack()
        try:
            pools = (
                ctx.enter_context(tc.tile_pool(name="consts", bufs=1)),
                ctx.enter_context(tc.tile_pool(name="big1", bufs=1)),
                ctx.enter_context(tc.tile_pool(name="big2", bufs=2)),
                ctx.enter_context(tc.tile_pool(name="ps_big", bufs=4, space="PSUM")),
                ctx.enter_context(tc.tile_pool(name="ps_row", bufs=1, space="PSUM")),
                ctx.enter_context(tc.tile_pool(name="ps_sm", bufs=1, space="PSUM")),
                ctx.enter_context(tc.tile_pool(name="fm", bufs=1)),
                ctx.enter_context(tc.tile_pool(name="vec", bufs=6)),
                ctx.enter_context(tc.tile_pool(name="glue", bufs=4)),
            )
            for _rep in range(repeat):
                emit(tc, ctx, pools)
        finally:
            ctx.close()

    nc.compile()
    nc.finalize()
    return nc


def _host_inputs(inputs):
    """Map the oracle's inputs to the kernel's DRAM parameters."""

    def f(k):
        return np.ascontiguousarray(np.asarray(inputs[k], dtype=np.float32))

    m = {"hs": f("hs")}
    dis = np.zeros(8192, np.float32)
    dis[:E] = np.asarray(inputs["dis"], np.float32).reshape(-1)
    m["dis"] = dis
    off = 3 * RES
    m["e0c"] = f("E0")[off : off + N * RES].copy()
    for pre in ("n", "c", "k", "e"):
        for nm in ("W1", "W2", "W3", "b1", "b2", "b3"):
            m[pre + nm] = f(pre + nm)
    sdn = np.zeros((128, 128), np.float32)
    sup = np.zeros((128, 128), np.float32)
    for q in range(127):
        sdn[q + 1, q] = 1.0  # lhsT: out[m] = v[m+1]
        sup[q, q + 1] = 1.0  # lhsT: out[m] = v[m-1]
    m["sdn"] = sdn
    m["sup"] = sup
    bmask = np.ones((128, 64), np.float32)
    bmask[0, 0] = bmask[0, 1] = 0.0        # band o=-2: rows 0,1 invalid
    bmask[0, 16] = 0.0                     # band o=-1: row 0 invalid
    bmask[127, 32 + 15] = 0.0              # band o=+1: row 2047 invalid
    bmask[127, 48 + 14] = bmask[127, 48 + 15] = 0.0  # band o=+2: rows 2046,2047
    m["bmask"] = bmask
    return m


def kernel(**inputs):
    from concourse.bass_utils import run_bass_kernel_spmd

    src = np.asarray(inputs["src"])
    for o, i0, L, e0 in BANDS:
        assert src[e0] == i0 and src[e0 + L - 1] == i0 + L - 1, "unexpected edge order"

    if "nc" not in _CACHE:
        _CACHE["nc"] = _build()
    nc = _CACHE["nc"]

    m = _host_inputs(inputs)
    res = run_bass_kernel_spmd(nc, [m] * 8, core_ids=list(range(8)))
    out = res.results[0]["out"]  # [N*RES, 2] float32
    en = out[:, 0].astype(np.float32) + 1j * out[:, 1].astype(np.float32)
    return en.astype(np.complex64)



# revision 3
# speedup vs baseline: 2.6395x; 2.6395x over previous
"""Trainium2 Bass kernel for nn_Metalayer_sub_62869731279045.

Math: the oracle's edge list is the structured 1-D KNN=2 neighbor graph, so
C = I + Delta and Km are pentadiagonal.  Let D = -Delta and

  G  = wh * (B C + K)            (pentadiagonal, row-diagonals g_o)
  Ninv ~= I + D + D^2 + D^3      (Neumann, band 6)
  T  = Ninv * G - theta*I        (band 8, 17 diagonals, built on device
                                  via banded matrix-matrix products)

Since T is REAL, the expm action needs only a real Taylor chain:
  tau_k = T^k u0,   Uz = e^{i theta} * DX * sum_k (i^k / k!) tau_k
with i^k folded into which accumulator (s_re / s_im) receives each term.
KT=5 terms + fp16 MLP staging give ~1.2e-3 max-rel error vs fp64.

Layouts: length-2048 row vectors live as [128, 16] "fm" tiles (i = 16p+f).
Banded matvec = halo exchange via 2 PE shift-matmuls + one DVE windowed
multiply against 17 stacked diagonal planes + one segmented reduce.
Banded products for the operator assembly use the same windowed-multiply
trick over zero-padded plane tiles (all strides positive by storing the
5-wide D/G plane stacks in reversed diagonal order).

MLPs: c&k edge MLPs run as one 128-wide hidden pipeline over 8192 stacked
edge columns (4 bands); n&e node MLPs as one 128-wide pipeline whose last
layer emits Bd and all 32 Eys rows from a single [128,34] matmul.  Row ->
fm transposes bounce through DRAM with 64B-run descriptor patterns.

All 8 cores run the same single-core program on identical inputs (the
chain is a serial dependency; collectives cost more than they save).
Core 0's output is returned.
"""

import os
import sys
import numpy as np

for _p in ("/opt/trn_rl_repo",):
    if _p not in sys.path:
        sys.path.insert(0, _p)

N = 2048
RES = 32
H = 64
E = 8186
K_WAVE = 2.0 * np.pi / 1.55
WH = 0.75
DX = 1.0 / RES
THETA = 6.234
KT = 5    # Taylor terms
# band order for the stacked ck pipeline: o descending (matches reversed
# diagonal-plane storage so all product access patterns have +strides)
BAND_ORDER = [2, 1, -1, -2]
# (offset o, first valid row i0, edge count L, edge-array start e0)
BANDS = {-2: (2, 2046, 0), -1: (1, 2047, 2046), 1: (0, 2047, 4093), 2: (0, 2046, 6140)}

_CACHE = {}


def _build():
    from contextlib import ExitStack

    import concourse.bass as bass
    import concourse.mybir as mybir
    from concourse import bacc, tile

    f32 = mybir.dt.float32
    f16 = mybir.dt.float16
    AF = mybir.ActivationFunctionType
    ALU = mybir.AluOpType

    nc = bacc.Bacc("TRN2", target_bir_lowering=False, debug=False, num_devices=8)

    def Par(name, shape, dt=f32):
        return nc.declare_dram_parameter(name, list(shape), dt, isOutput=False)

    hs16_d = Par("hs16", [1, N], f16)
    xt16_d = Par("xt16", [3, 4 * N], f16)
    W1ne_d = Par("W1ne", [1, 128], f16)
    W2ne_d = Par("W2ne", [128, 128], f16)
    W3ne_d = Par("W3ne", [128, 34], f16)
    b1ne_d = Par("b1ne", [128, 1])
    b2ne_d = Par("b2ne", [128, 1])
    W1ck_d = Par("W1ck", [3, 128], f16)
    W2ck_d = Par("W2ck", [128, 128], f16)
    W3ck_d = Par("W3ck", [128, 2], f16)
    b1ck_d = Par("b1ck", [128, 1])
    b2ck_d = Par("b2ck", [128, 1])
    b3ck_d = Par("b3ckrep", [128, 2])
    nb3_d = Par("nb3rep", [128, 1])
    eb3_d = Par("eb3rep", [128, RES])
    e0c_d = Par("e0c", [N * RES])
    sdn_d = Par("sdn", [128, 128])
    sup_d = Par("sup", [128, 128])
    mask_d = Par("bmask", [128, 64])
    scratch = nc.dram_tensor("scratch", [43 * N], f32)
    out_d = nc.declare_dram_parameter("out", [N * RES, 2], f32, isOutput=True)

    TS = bass.ts

    def emit(tc, ctx, pools):
        (consts, work, vec, psA, psB, psH) = pools

        def dma(out_ap, in_ap, eng=None):
            (eng or nc.gpsimd).dma_start(out_ap, in_ap)

        # ---------------- constants / weights (issue all up front) ----------
        def load(name, dram, shape, dt=f32, eng=None):
            t = consts.tile(list(shape), dt, tag=name)
            dma(t[:], dram[:], eng)
            return t

        hs16 = load("hs16", hs16_d, (1, N), f16)
        xt16 = load("xt16", xt16_d, (3, 4 * N), f16, eng=nc.sync)
        W1ne = load("W1ne", W1ne_d, (1, 128), f16)
        W2ne = load("W2ne", W2ne_d, (128, 128), f16, eng=nc.sync)
        W3ne = load("W3ne", W3ne_d, (128, 34), f16)
        b1ne = load("b1ne", b1ne_d, (128, 1))
        b2ne = load("b2ne", b2ne_d, (128, 1), eng=nc.sync)
        W1ck = load("W1ck", W1ck_d, (3, 128), f16)
        W2ck = load("W2ck", W2ck_d, (128, 128), f16, eng=nc.sync)
        W3ck = load("W3ck", W3ck_d, (128, 2), f16)
        b1ck = load("b1ck", b1ck_d, (128, 1))
        b2ck = load("b2ck", b2ck_d, (128, 1), eng=nc.sync)
        b3ck = load("b3ck", b3ck_d, (128, 2))
        nb3 = load("nb3", nb3_d, (128, 1))
        eb3 = load("eb3", eb3_d, (128, RES), eng=nc.sync)
        e0c = load("e0c", e0c_d, (128, 16 * RES))
        sdn = load("sdn", sdn_d, (128, 128), eng=nc.sync)
        sup = load("sup", sup_d, (128, 128))
        bmask = load("bmask", mask_d, (128, 64), eng=nc.sync)

        # ---------------- persistent SBUF tiles ----------------
        h1ne = consts.tile([128, N], f16, tag="h1ne")
        h2ne = consts.tile([128, N], f16, tag="h2ne")
        rows_ne = consts.tile([34, N], f32, tag="rows_ne")
        h1ck = consts.tile([128, 4 * N], f16, tag="h1ck")
        h2ck = consts.tile([128, 4 * N], f16, tag="h2ck")
        rows_ck = consts.tile([2, 4 * N], f32, tag="rows_ck")
        ckbdfm = consts.tile([128, 144], f32, tag="ckbdfm")
        eys2 = consts.tile([128, 16 * RES], f32, tag="eys2")
        eys2b = consts.tile([128, 16 * RES], f32, tag="eys2b")
        u0 = consts.tile([128, 16], f32, tag="u0")
        Dt = consts.tile([128, 80], f32, tag="Dt")        # 5 planes x 16, rev
        Gt = consts.tile([128, 200], f32, tag="Gt")       # 5 planes x 40, rev
        N1p = consts.tile([128, 260], f32, tag="N1p")     # 13 planes x 20
        N2p = consts.tile([128, 340], f32, tag="N2p")     # 17 planes x 20
        N3p = consts.tile([128, 420], f32, tag="N3p")     # 21 planes x 20
        Tpl = consts.tile([128, 272], f32, tag="Tpl")     # col = f*17 + s
        s_re = consts.tile([128, 16], f32, tag="s_re")
        s_im = consts.tile([128, 16], f32, tag="s_im")
        o_int = consts.tile([128, 2 * 16 * RES], f32, tag="o_int")

        AP = bass.AP

        def ap(t, off, dims):
            return AP(t.tensor, t.offset + off, dims)

        # early memsets of padded plane tiles (no deps -> overlap with MLPs)
        nc.vector.memset(Gt[:], 0.0)
        nc.gpsimd.memset(N1p[:], 0.0)
        nc.vector.memset(N2p[:], 0.0)
        nc.gpsimd.memset(N3p[:], 0.0)
        nc.vector.memset(Dt[:, 32:48], 0.0)
        nc.gpsimd.memset(s_im[:], 0.0)

        # ---------------- ne pipeline (n & e MLPs, 2048 cols) ----------------
        for q in range(4):
            ps1 = psA.tile([128, 512], f32, tag="psA")
            nc.tensor.matmul(ps1[:], W1ne[:], hs16[:, TS(q, 512)])
            nc.gpsimd.tensor_scalar(
                h1ne[:, TS(q, 512)], ps1[:], b1ne[:], 0.0, ALU.add, op1=ALU.max
            )
            ps2 = psA.tile([128, 512], f32, tag="psA")
            nc.tensor.matmul(ps2[:], W2ne[:], h1ne[:, TS(q, 512)])
            nc.gpsimd.tensor_scalar(
                h2ne[:, TS(q, 512)], ps2[:], b2ne[:], 0.0, ALU.add, op1=ALU.max
            )
            ps3 = psA.tile([128, 512], f32, tag="psA")
            nc.tensor.matmul(ps3[0:34, :], W3ne[:], h2ne[:, TS(q, 512)])
            nc.vector.tensor_copy(rows_ne[:, TS(q, 512)], ps3[0:34, :])

        # ne rows -> DRAM scratch (row 0: Bd_pre; rows 11..43: eys)
        dma(AP(scratch, 0, [[1, N]]), rows_ne[0:1, :], eng=nc.sync)
        dma(AP(scratch, 11 * N, [[2048, 32], [1, N]]), rows_ne[2:34, :], eng=nc.sync)
        # eys2[p, r*16+f] = eys[16p+f, r]
        dma(
            ap(eys2, 0, [[512, 128], [16, 32], [1, 16]]),
            AP(scratch, 11 * N, [[16, 128], [2048, 32], [1, 16]]),
        )
        # eys2b = eys2 + eb3 (per-r bias)
        nc.vector.tensor_tensor(
            ap(eys2b, 0, [[512, 128], [16, 32], [1, 16]]),
            ap(eys2, 0, [[512, 128], [16, 32], [1, 16]]),
            ap(eb3, 0, [[RES, 128], [1, 32], [0, 16]]),
            ALU.add,
        )
        # u0[i] = sum_r eys2b[i,r] * e0c[i,r]
        pu = work.tile([128, 16 * RES], f32, tag="pu")
        nc.vector.tensor_tensor(
            ap(pu, 0, [[512, 128], [32, 16], [1, 32]]),
            ap(eys2b, 0, [[512, 128], [1, 16], [16, 32]]),
            ap(e0c, 0, [[512, 128], [32, 16], [1, 32]]),
            ALU.mult,
        )
        nc.vector.reduce_sum(
            u0[:],
            ap(pu, 0, [[512, 128], [32, 16], [1, 32]]),
            axis=mybir.AxisListType.X,
        )

        # ---------------- ck pipeline (c & k edge MLPs, 4*2048 cols) ---------
        for q in range(16):
            ps1 = psA.tile([128, 512], f32, tag="psA")
            nc.tensor.matmul(ps1[:], W1ck[:], xt16[:, TS(q, 512)])
            nc.scalar.activation(
                h1ck[:, TS(q, 512)], ps1[:], AF.Relu, bias=b1ck[:]
            )
            ps2 = psA.tile([128, 512], f32, tag="psA")
            nc.tensor.matmul(ps2[:], W2ck[:], h1ck[:, TS(q, 512)])
            nc.vector.tensor_scalar(
                h2ck[:, TS(q, 512)], ps2[:], b2ck[:], 0.0, ALU.add, op1=ALU.max
            )
            if q % 2 == 0:
                ps3 = psB.tile([2, 1024], f32, tag="psB")
            nc.tensor.matmul(
                ps3[:, TS(q % 2, 512)], W3ck[:], h2ck[:, TS(q, 512)]
            )
            if q % 2 == 1:
                eng = nc.scalar if (q // 2) % 2 == 0 else nc.gpsimd
                if eng is nc.scalar:
                    nc.scalar.activation(
                        rows_ck[:, TS(q // 2, 1024)], ps3[:], AF.Copy
                    )
                else:
                    nc.gpsimd.tensor_copy(rows_ck[:, TS(q // 2, 1024)], ps3[:])

        # ck rows -> scratch rows 1..9, then one strided fm readback
        dma(AP(scratch, N, [[8192, 2], [1, 8192]]), rows_ck[:], eng=nc.sync)
        # ckbdfm[p, g*16+f] = scratch[g*2048 + 16p+f]; g0=Bd_pre, g1..4=c bands,
        # g5..8=k bands (band order +2,+1,-1,-2)
        dma(
            ap(ckbdfm, 0, [[144, 128], [16, 9], [1, 16]]),
            AP(scratch, 0, [[16, 128], [2048, 9], [1, 16]]),
        )

        # ---------------- diagonal planes ----------------
        tanhc = work.tile([128, 64], f32, tag="tanhc")
        tanhk = work.tile([128, 64], f32, tag="tanhk")
        tb = work.tile([128, 16], f32, tag="tb")
        Bdp = work.tile([128, 16], f32, tag="Bdp")
        Bdp01 = work.tile([128, 16], f32, tag="Bdp01")
        nc.scalar.activation(tanhc[:], ckbdfm[:, 16:80], AF.Tanh, bias=b3ck[:, 0:1])
        nc.scalar.activation(tanhk[:], ckbdfm[:, 80:144], AF.Tanh, bias=b3ck[:, 1:2])
        nc.scalar.activation(tb[:], ckbdfm[:, 0:16], AF.Tanh, bias=nb3[:])
        # Bd' = wh*K*(2 + 0.5*tanh)
        nc.vector.tensor_scalar(
            Bdp[:], tb[:], 0.5 * K_WAVE * WH, 2.0 * K_WAVE * WH, ALU.mult, op1=ALU.add
        )
        nc.vector.tensor_scalar(Bdp01[:], Bdp[:], 0.1, 0.0, ALU.mult, op1=ALU.add)

        # D planes (reversed: plane j <-> o1 = 2-j), d = -0.1*tanh_c*mask
        nc.vector.scalar_tensor_tensor(
            ap(Dt, 0, [[80, 128], [16, 2], [1, 16]]),
            ap(tanhc, 0, [[64, 128], [16, 2], [1, 16]]),
            -0.1,
            ap(bmask, 0, [[64, 128], [16, 2], [1, 16]]),
            ALU.mult,
            ALU.mult,
        )
        nc.vector.scalar_tensor_tensor(
            ap(Dt, 48, [[80, 128], [16, 2], [1, 16]]),
            ap(tanhc, 32, [[64, 128], [16, 2], [1, 16]]),
            -0.1,
            ap(bmask, 32, [[64, 128], [16, 2], [1, 16]]),
            ALU.mult,
            ALU.mult,
        )
        # G planes (reversed, width 40 = 12|16|12), g = (0.1*Bd'*tc + ck2*tk)*mask
        gm4 = work.tile([128, 64], f32, tag="gm4")
        g4 = work.tile([128, 64], f32, tag="g4")
        nc.vector.tensor_tensor(
            ap(gm4, 0, [[64, 128], [16, 4], [1, 16]]),
            ap(tanhc, 0, [[64, 128], [16, 4], [1, 16]]),
            ap(Bdp01, 0, [[16, 128], [0, 4], [1, 16]]),
            ALU.mult,
        )
        nc.vector.scalar_tensor_tensor(
            g4[:], tanhk[:], 0.1 * K_WAVE * WH, gm4[:], ALU.mult, ALU.add
        )
        nc.vector.tensor_tensor(
            ap(Gt, 12, [[200, 128], [40, 2], [1, 16]]),
            ap(g4, 0, [[64, 128], [16, 2], [1, 16]]),
            ap(bmask, 0, [[64, 128], [16, 2], [1, 16]]),
            ALU.mult,
        )
        nc.vector.tensor_tensor(
            ap(Gt, 3 * 40 + 12, [[200, 128], [40, 2], [1, 16]]),
            ap(g4, 32, [[64, 128], [16, 2], [1, 16]]),
            ap(bmask, 32, [[64, 128], [16, 2], [1, 16]]),
            ALU.mult,
        )
        nc.vector.tensor_copy(Gt[:, 2 * 40 + 12 : 2 * 40 + 28], Bdp[:])
        # N1 = I + D, ascending planes 4..8 (idx = o+6), width 20 = 2|16|2
        for j, b0 in ((4, 48), (5, 32), (7, 16), (8, 0)):
            nc.vector.scalar_tensor_tensor(
                ap(N1p, j * 20 + 2, [[260, 128], [1, 16]]),
                ap(tanhc, b0, [[64, 128], [1, 16]]),
                -0.1,
                ap(bmask, b0, [[64, 128], [1, 16]]),
                ALU.mult,
                ALU.mult,
            )
        nc.vector.memset(N1p[:, 6 * 20 + 2 : 6 * 20 + 18], 1.0)

        # halo fill helper: data planes [first..first+n) of a padded tile,
        # plane width w, pad width hw, data width 16.
        def halo(t, first, nplanes, w, hw):
            ps = psH.tile([128, 2 * nplanes * hw], f32, tag="psH")
            base = first * w + hw
            # left pads <- v[p-1, f in 16-hw..16] (sup)
            nc.tensor.matmul(
                ps[:, 0 : nplanes * hw],
                sup[:],
                ap(t, base + 16 - hw, [[t.shape[1], 128], [w, nplanes], [1, hw]]),
            )
            # right pads <- v[p+1, f in 0..hw] (sdn)
            nc.tensor.matmul(
                ps[:, nplanes * hw : 2 * nplanes * hw],
                sdn[:],
                ap(t, base, [[t.shape[1], 128], [w, nplanes], [1, hw]]),
            )
            nc.vector.tensor_copy(
                ap(
                    t,
                    first * w,
                    [[t.shape[1], 128], [16 + hw, 2], [w, nplanes], [1, hw]],
                ),
                ap(
                    ps,
                    0,
                    [[2 * nplanes * hw, 128], [nplanes * hw, 2], [hw, nplanes], [1, hw]],
                ),
            )

        halo(Gt, 0, 5, 40, 6)
        halo(N1p, 4, 5, 20, 2)

        # banded product: out_pad.data = D * in_pad  (planes ascending)
        # in0 = Dt (reversed), in1 = in_pad at addr 20*O + 19*j + f + 4
        def dprod(in_pad, out_pad, NPo, tag):
            pr = work.tile([128, NPo * 80], f32, tag=tag)
            pr_ap = ap(pr, 0, [[NPo * 80, 128], [80, NPo], [5, 16], [1, 5]])
            nc.vector.tensor_tensor(
                pr_ap,
                ap(Dt, 0, [[80, 128], [0, NPo], [1, 16], [16, 5]]),
                ap(in_pad, 4, [[in_pad.shape[1], 128], [20, NPo], [1, 16], [19, 5]]),
                ALU.mult,
            )
            nc.vector.reduce_sum(
                ap(out_pad, 82, [[out_pad.shape[1], 128], [20, NPo], [1, 16]]),
                pr_ap,
                axis=mybir.AxisListType.X,
            )

        dprod(N1p, N2p, 9, "pr2")
        nc.vector.tensor_scalar(
            N2p[:, 8 * 20 + 2 : 8 * 20 + 18],
            N2p[:, 8 * 20 + 2 : 8 * 20 + 18],
            1.0, 0.0, ALU.add, op1=ALU.add,
        )
        halo(N2p, 4, 9, 20, 2)
        dprod(N2p, N3p, 13, "pr3")
        nc.vector.tensor_scalar(
            N3p[:, 10 * 20 + 2 : 10 * 20 + 18],
            N3p[:, 10 * 20 + 2 : 10 * 20 + 18],
            1.0, 0.0, ALU.add, op1=ALU.add,
        )
        # T = N3 * G - theta I   (no halo needed on N3p: read unshifted)
        prT = work.tile([128, 17 * 80], f32, tag="prT")
        prT_ap = ap(prT, 0, [[17 * 80, 128], [80, 17], [5, 16], [1, 5]])
        nc.vector.tensor_tensor(
            prT_ap,
            ap(N3p, 2, [[420, 128], [20, 17], [1, 16], [20, 5]]),
            ap(Gt, 2, [[200, 128], [1, 17], [1, 16], [41, 5]]),
            ALU.mult,
        )
        nc.vector.reduce_sum(
            ap(Tpl, 0, [[272, 128], [1, 17], [17, 16]]),
            prT_ap,
            axis=mybir.AxisListType.X,
        )
        nc.vector.tensor_scalar(
            ap(Tpl, 8, [[272, 128], [17, 16]]),
            ap(Tpl, 8, [[272, 128], [17, 16]]),
            -THETA, 0.0, ALU.add, op1=ALU.add,
        )

        # ---------------- real Taylor chain ----------------
        t_cur = vec.tile([128, 32], f32, tag="vec")
        nc.vector.memset(t_cur[:], 0.0)
        nc.vector.tensor_copy(t_cur[:, 8:24], u0[:])
        nc.vector.tensor_scalar(
            s_re[:], u0[:], DX, 0.0, ALU.mult, op1=ALU.add
        )
        fact = 1.0
        for k in range(1, KT + 1):
            psh = psH.tile([128, 16], f32, tag="psH")
            nc.tensor.matmul(psh[:, 0:8], sup[:], t_cur[:, 16:24])
            nc.tensor.matmul(psh[:, 8:16], sdn[:], t_cur[:, 8:16])
            nc.vector.tensor_copy(
                ap(t_cur, 0, [[32, 128], [24, 2], [1, 8]]),
                ap(psh, 0, [[16, 128], [8, 2], [1, 8]]),
            )
            pr = work.tile([128, 272], f32, tag="prc")
            pr_ap = ap(pr, 0, [[272, 128], [17, 16], [1, 17]])
            nc.vector.tensor_tensor(
                pr_ap,
                ap(t_cur, 0, [[32, 128], [1, 16], [1, 17]]),
                ap(Tpl, 0, [[272, 128], [17, 16], [1, 17]]),
                ALU.mult,
            )
            t_nxt = vec.tile([128, 32], f32, tag="vec")
            nc.vector.reduce_sum(
                ap(t_nxt, 8, [[32, 128], [1, 16]]), pr_ap, axis=mybir.AxisListType.X
            )
            fact *= k
            coef = DX / fact * (-1.0 if k % 4 in (2, 3) else 1.0)
            dst = s_im if k % 2 == 1 else s_re
            nc.vector.scalar_tensor_tensor(
                dst[:], t_nxt[:, 8:24], coef, dst[:], ALU.mult, ALU.add
            )
            t_cur = t_nxt

        # ---------------- Uz = e^{i theta} s;  En = Uz * Eys ----------------
        cth, sth = float(np.cos(THETA)), float(np.sin(THETA))
        uzr = work.tile([128, 16], f32, tag="uzr")
        uzi = work.tile([128, 16], f32, tag="uzi")
        p1 = work.tile([128, 16], f32, tag="p1")
        nc.vector.tensor_scalar(p1[:], s_im[:], sth, 0.0, ALU.mult, op1=ALU.add)
        nc.vector.scalar_tensor_tensor(
            uzr[:], s_re[:], cth, p1[:], ALU.mult, ALU.subtract
        )
        nc.vector.tensor_scalar(p1[:], s_re[:], sth, 0.0, ALU.mult, op1=ALU.add)
        nc.vector.scalar_tensor_tensor(
            uzi[:], s_im[:], cth, p1[:], ALU.mult, ALU.add
        )
        for c, uz in ((0, uzr), (1, uzi)):
            nc.vector.tensor_tensor(
                ap(o_int, c, [[1024, 128], [64, 16], [2, 32]]),
                ap(eys2b, 0, [[512, 128], [1, 16], [16, 32]]),
                ap(uz, 0, [[16, 128], [1, 16], [0, 32]]),
                ALU.mult,
            )
        nc.sync.dma_start(AP(out_d, 0, [[1024, 128], [1, 1024]]), o_int[:])

    with tile.TileContext(nc) as tc:
        ctx = ExitStack()
        try:
            pools = (
                ctx.enter_context(tc.tile_pool(name="consts", bufs=1)),
                ctx.enter_context(tc.tile_pool(name="work", bufs=2)),
                ctx.enter_context(tc.tile_pool(name="vec", bufs=3)),
                ctx.enter_context(tc.tile_pool(name="psA", bufs=3, space="PSUM")),
                ctx.enter_context(tc.tile_pool(name="psB", bufs=2, space="PSUM")),
                ctx.enter_context(tc.tile_pool(name="psH", bufs=1, space="PSUM")),
            )
            emit(tc, ctx, pools)
        finally:
            ctx.close()

    nc.compile()
    nc.finalize()
    return nc


def _host_inputs(inputs):
    """Stage the oracle's inputs into the kernel's DRAM parameters."""
    f16 = np.float16

    def f(k):
        return np.ascontiguousarray(np.asarray(inputs[k], dtype=np.float32))

    hs = f("hs")
    dis = f("dis").reshape(-1)
    m = {"hs16": hs.astype(f16)[None, :]}

    xt = np.zeros((3, 4 * N), np.float32)
    bmask = np.zeros((128, 64), np.float32)
    for b, o in enumerate(BAND_ORDER):
        i0, L, e0 = BANDS[o]
        i = np.arange(i0, i0 + L)
        xt[0, b * N + i] = hs[i]
        xt[1, b * N + i] = hs[i + o]
        xt[2, b * N + i] = dis[e0 : e0 + L]
        bm = np.zeros(N, np.float32)
        bm[i] = 1.0
        bmask[:, b * 16 : (b + 1) * 16] = bm.reshape(128, 16)
    m["xt16"] = xt.astype(f16)
    m["bmask"] = bmask

    def blockdiag(a, b):
        z = np.zeros((128, 128), np.float32)
        z[0:64, 0:64] = a
        z[64:128, 64:128] = b
        return z

    m["W1ne"] = np.concatenate([f("nW1"), f("eW1")], axis=1).astype(f16)
    m["W2ne"] = blockdiag(f("nW2"), f("eW2")).astype(f16)
    w3ne = np.zeros((128, 34), np.float32)
    w3ne[0:64, 0:1] = f("nW3")
    w3ne[64:128, 2:34] = f("eW3")
    m["W3ne"] = w3ne.astype(f16)
    m["b1ne"] = np.concatenate([f("nb1"), f("eb1")])[:, None]
    m["b2ne"] = np.concatenate([f("nb2"), f("eb2")])[:, None]
    m["W1ck"] = np.concatenate([f("cW1"), f("kW1")], axis=1).astype(f16)
    m["W2ck"] = blockdiag(f("cW2"), f("kW2")).astype(f16)
    w3ck = np.zeros((128, 2), np.float32)
    w3ck[0:64, 0:1] = f("cW3")
    w3ck[64:128, 1:2] = f("kW3")
    m["W3ck"] = w3ck.astype(f16)
    m["b1ck"] = np.concatenate([f("cb1"), f("kb1")])[:, None]
    m["b2ck"] = np.concatenate([f("cb2"), f("kb2")])[:, None]
    m["b3ckrep"] = np.tile(
        np.array([f("cb3")[0], f("kb3")[0]], np.float32)[None, :], (128, 1)
    )
    m["nb3rep"] = np.tile(f("nb3").reshape(1, 1), (128, 1))
    m["eb3rep"] = np.tile(f("eb3")[None, :], (128, 1))

    off = 3 * RES
    m["e0c"] = f("E0")[off : off + N * RES].copy()

    sdn = np.zeros((128, 128), np.float32)
    sup = np.zeros((128, 128), np.float32)
    for q in range(127):
        sdn[q + 1, q] = 1.0  # lhsT: out[m] = v[m+1]
        sup[q, q + 1] = 1.0  # lhsT: out[m] = v[m-1]
    m["sdn"] = sdn
    m["sup"] = sup
    return m


def kernel(**inputs):
    from concourse.bass_utils import run_bass_kernel_spmd

    src = np.asarray(inputs["src"])
    for o, (i0, L, e0) in BANDS.items():
        assert src[e0] == i0 and src[e0 + L - 1] == i0 + L - 1, "unexpected edge order"

    if "nc" not in _CACHE:
        _CACHE["nc"] = _build()
    nc = _CACHE["nc"]

    m = _host_inputs(inputs)
    res = run_bass_kernel_spmd(nc, [m] * 8, core_ids=list(range(8)))
    out = res.results[0]["out"]  # [N*RES, 2] float32
    en = out[:, 0].astype(np.float32) + 1j * out[:, 1].astype(np.float32)
    return en.astype(np.complex64)
